# revision 22
# baseline (speedup 1.0000x reference)
"""Trainium2 Bass kernel for nn_Encoder (B=4, S=2048, D=512, H=8 self-attention).

Sharding over 8 NeuronCores: core c -> (batch b = c//2, head-group hg = c%2).
Each core computes, for its batch and its 4 heads, the full attention block
plus a partial output projection y_part = attn_out @ Wo[group rows]. The host
sums the two partial y tensors per batch (the head-concat + Wo projection is
linear in the head groups).

Key compaction: the key-padding mask zeroes ~half the keys exactly
(exp(-1e9) == 0 in f32), so the host gathers only the unmasked keys per batch
and pads to SK (multiple of 128). Padded keys get kt=0 (score 0) and
mbias=-1e9, so their probs are exactly 0 — identical math, ~44% less
scores/exp/attnV work.

Device-side layout (everything transposed so the contraction dim is always on
SBUF partitions):
  xT [D, S]         : host-pretransposed input, d on partitions (4 chunks)
  xkT [D, SK]       : compacted keys' input columns
  KT per pair       : [128, SK] = [2 heads' e, s], from W.T @ xk matmuls
  QT per pair       : [128, S]
  scoresT [s, t]    : s on partitions -> key-padding mask becomes a
                      per-partition bias AP fused into the ACT Exp instruction
                      (scale=1/sqrt(DH) fused there too)
  V' [s, e + ones]  : appended ones column makes the softmax denominator fall
                      out of the attnV matmul (psum row 64) for free
  outT [he, t]      : exactly the lhsT layout the Wo projection wants
"""

import ml_dtypes
import numpy as np

import concourse.mybir as mybir
import concourse.tile as tile
from concourse import bacc
from concourse.bass_utils import run_bass_kernel_spmd

B, S, D, H = 4, 2048, 512, 8
DH = D // H          # 64
HPC = H // 2         # 4 heads per core
HE = HPC * DH        # 256 output-proj rows per core
T = S                # full query length per core
NDC = D // 128       # 4 contraction chunks for projections
MASK_NUM = 1.0e9
N_CORES = 8

f32 = mybir.dt.float32
bf16 = mybir.dt.bfloat16
EXP = mybir.ActivationFunctionType.Exp


def _chunks(total, width):
    out = []
    o = 0
    while o < total:
        w = min(width, total - o)
        out.append((o, w))
        o += w
    return out


def build_nc(SK):
    NSTK = SK // 128     # key tiles
    nc = bacc.Bacc("TRN2", target_bir_lowering=False, debug=False, num_devices=1)

    xT = nc.dram_tensor("xT", [D, S], bf16, kind="ExternalInput").ap()
    xkT = nc.dram_tensor("xkT", [D, SK], bf16, kind="ExternalInput").ap()
    wq = nc.dram_tensor("wq", [D, HE], bf16, kind="ExternalInput").ap()
    wk = nc.dram_tensor("wk", [D, HE], bf16, kind="ExternalInput").ap()
    wv = nc.dram_tensor("wv", [D, HE], bf16, kind="ExternalInput").ap()
    wo = nc.dram_tensor("wo", [HE, D], bf16, kind="ExternalInput").ap()
    mb = nc.dram_tensor("mbias", [SK], f32, kind="ExternalInput").ap()
    y = nc.dram_tensor("y", [T, D], f32, kind="ExternalOutput").ap()

    with tile.TileContext(nc) as tc:
        with (
            tc.tile_pool(name="const", bufs=1) as const,
            tc.tile_pool(name="psA", bufs=4, space="PSUM") as psA,
            tc.tile_pool(name="psS", bufs=2, space="PSUM") as psS,
            tc.tile_pool(name="attnT", bufs=14) as at_pool,
            tc.tile_pool(name="yout", bufs=3) as y_pool,
            tc.tile_pool(name="recip", bufs=4) as r_pool,
            tc.tile_pool(name="recipb", bufs=4) as rb_pool,
            tc.tile_pool(name="avsb", bufs=8) as av_pool,
            tc.tile_pool(name="sums", bufs=2) as sums_pool,
        ):
            # ---- Stage A: loads -------------------------------------------
            xT_sb = const.tile([128, NDC, S], bf16, tag="xT")
            xk_sb = const.tile([128, NDC, SK], bf16, tag="xk")
            wq_sb = const.tile([128, NDC, HE], bf16, tag="wq")
            wk_sb = const.tile([128, NDC, HE], bf16, tag="wk")
            wv_sb = const.tile([128, NDC, HE], bf16, tag="wv")
            wo_sb = const.tile([128, HE // 128, D], bf16, tag="wo")
            mb_sb = const.tile([128, NSTK], f32, tag="mb")
            xT_r = xT.rearrange("(c p) s -> c p s", p=128)
            xk_r = xkT.rearrange("(c p) s -> c p s", p=128)
            # DMA order: everything the first scores tile + first EXP needs
            # lands first (wk, xk c0, wq, xT c0, mbias), then the rest in
            # roughly the order projection chunks consume it.
            nc.sync.dma_start(wk_sb[:], wk.rearrange("(c p) n -> p c n", p=128))
            for dc in range(NDC):
                nc.sync.dma_start(xk_sb[:, dc, 0:512], xk_r[dc][:, 0:512])
            nc.sync.dma_start(wq_sb[:], wq.rearrange("(c p) n -> p c n", p=128))
            nc.sync.dma_start(mb_sb[:], mb.rearrange("(j p) -> p j", p=128))
            for dc in range(NDC):
                nc.sync.dma_start(xT_sb[:, dc, 0:512], xT_r[dc][:, 0:512])
            nc.sync.dma_start(wv_sb[:], wv.rearrange("(c p) n -> p c n", p=128))
            for dc in range(NDC):
                nc.sync.dma_start(xT_sb[:, dc, 512:1024], xT_r[dc][:, 512:1024])
            for off, w in _chunks(SK, 512)[1:]:
                for dc in range(NDC):
                    nc.sync.dma_start(
                        xk_sb[:, dc, off : off + w], xk_r[dc][:, off : off + w]
                    )
            nc.sync.dma_start(wo_sb[:], wo.rearrange("(c p) n -> p c n", p=128))
            for off in (1024, 1536):
                for dc in range(NDC):
                    nc.sync.dma_start(
                        xT_sb[:, dc, off : off + 512], xT_r[dc][:, off : off + 512]
                    )

            # V' tiles: [s-tile][local head][DH + ones column]
            v_sb = const.tile([128, NSTK, HPC, DH + 1], bf16, tag="v")
            nc.gpsimd.memset(v_sb[:, :, :, DH : DH + 1], 1.0)

            # ---- Stage B: projections -------------------------------------
            # KT per pair pp: [128 (2 heads' e), SK]; QT: [128, S]
            kt_sb = [
                const.tile([128, SK], bf16, tag=f"kt{pp}", name=f"kt{pp}")
                for pp in range(2)
            ]
            qt_sb = [
                const.tile([128, S], bf16, tag=f"qt{pp}", name=f"qt{pp}")
                for pp in range(2)
            ]

            def proj_chunk(pp, which, off, w):
                w_sb, src, dst = (
                    (wk_sb, xk_sb, kt_sb[pp])
                    if which == 0
                    else (wq_sb, xT_sb, qt_sb[pp])
                )
                ps = psA.tile([128, 512], f32, tag="mm", name="proj_ps")
                for dc in range(NDC):
                    nc.tensor.matmul(
                        ps[:, 0:w],
                        lhsT=w_sb[:, dc, pp * 128 : (pp + 1) * 128],
                        rhs=src[:, dc, off : off + w],
                        start=(dc == 0),
                        stop=(dc == NDC - 1),
                    )
                nc.vector.tensor_copy(dst[:, off : off + w], ps[:, 0:w])

            def emit_proj(pp):
                # kt chunk 0 + qt chunks 0,1 unblock the first key tiles;
                # the rest trail and complete while the st loop runs.
                kc = _chunks(SK, 512)
                order = [(0, kc[0]), (1, (0, 512)), (1, (512, 512))]
                order += [(0, c) for c in kc[1:]]
                order += [(1, (1024, 512)), (1, (1536, 512))]
                for which, (off, w) in order:
                    proj_chunk(pp, which, off, w)

            emit_proj(0)

            # V groups: split NSTK tiles into 4 groups emitted at sts 0..3
            base, rem = divmod(NSTK, 4)
            gsz = [base + (1 if g < rem else 0) for g in range(4)]
            goff = [sum(gsz[:g]) for g in range(4)]

            def emit_v_group(g):
                for vst in range(goff[g], goff[g] + gsz[g]):
                    ps = psA.tile([128, HE], f32, tag="mm", name="v_ps")
                    for dc in range(NDC):
                        nc.tensor.matmul(
                            ps[:],
                            lhsT=xk_sb[:, dc, vst * 128 : (vst + 1) * 128],
                            rhs=wv_sb[:, dc, :],
                            start=(dc == 0),
                            stop=(dc == NDC - 1),
                        )
                    nc.vector.tensor_copy(
                        v_sb[:, vst, :, 0:DH],
                        ps[:].rearrange("p (h e) -> p h e", e=DH),
                    )

            # ---- Stage C: attention ---------------------------------------
            # outT [he, t] laid out as [128, 2, T]: chunk pp, rows h2*64.
            # Phase = (th, pp). Normalize + Wo of phase P are deferred into
            # phase P+1 so boundaries never stall PE or ACT. pp=1 projections
            # are emitted at the first phase boundary, off the prologue.
            outT_sb = const.tile([128, HE // 128, T], bf16, tag="outT")

            def emit_normalize(pend):
                th_, pp_, av_sbs_ = pend
                sums = sums_pool.tile([97, 512], f32, tag="sums", name="sums")
                nc.gpsimd.memset(sums[:], 1.0)
                for h2 in range(2):
                    for tw in range(2):
                        k = 32 * (h2 * 2 + tw)
                        nc.vector.tensor_copy(
                            sums[k : k + 1, :],
                            av_sbs_[(h2, tw)][DH : DH + 1, :],
                        )
                recips = sums_pool.tile([97, 512], f32, tag="recips", name="recips")
                nc.vector.reciprocal_approx_fast(recips[:], sums[:])
                for h2 in range(2):
                    for tw in range(2):
                        k = 32 * (h2 * 2 + tw)
                        tcol = th_ * 1024 + tw * 512
                        r_t = r_pool.tile([1, 512], f32, tag="r", name="r_t")
                        nc.vector.tensor_copy(r_t[0:1, :], recips[k : k + 1, :])
                        rb_t = rb_pool.tile([64, 512], f32, tag="rb", name="rb_t")
                        nc.gpsimd.partition_broadcast(rb_t[:], r_t[0:1, :])
                        nc.vector.tensor_mul(
                            outT_sb[h2 * 64 : (h2 + 1) * 64, pp_, tcol : tcol + 512],
                            av_sbs_[(h2, tw)][0:DH, :],
                            rb_t[:],
                        )

            def emit_wo(th_, half=None):
                tts = range(th_ * 8, (th_ + 1) * 8)
                if half is not None:
                    tts = tts[half * 4 : half * 4 + 4]
                for tt in tts:
                    ps = psA.tile([128, 512], f32, tag="mm", name="y_ps")
                    for c in range(HE // 128):
                        nc.tensor.matmul(
                            ps[:],
                            lhsT=outT_sb[:, c, tt * 128 : (tt + 1) * 128],
                            rhs=wo_sb[:, c, :],
                            start=(c == 0),
                            stop=(c == HE // 128 - 1),
                        )
                    y_sb = y_pool.tile([128, 512], f32, tag="y", name="y_sb")
                    nc.vector.tensor_copy(y_sb[:], ps[:])
                    nc.sync.dma_start(y[tt * 128 : (tt + 1) * 128, :], y_sb[:])

            KC = _chunks(SK, 512)
            pending = None
            phases = [(th, pp) for th in range(T // 1024) for pp in range(2)]
            for phase_i, (th, pp) in enumerate(phases):
                defer_v = phase_i == 0
                defer_av = phase_i == 1
                av = None
                if not defer_v and not defer_av:
                    av = [
                        [
                            psA.tile([128, 512], f32, tag="mm", name=f"av{h2}_{tw}")
                            for tw in range(2)
                        ]
                        for h2 in range(2)
                    ]
                deferred_at = []

                def emit_attnv(at_, st_, h2_):
                    h = 2 * pp + h2_
                    for tw in range(2):
                        nc.tensor.matmul(
                            av[h2_][tw][0 : DH + 1, :],
                            lhsT=v_sb[:, st_, h, :],
                            rhs=at_[:, tw * 512 : (tw + 1) * 512],
                            start=(st_ == 0),
                            stop=(st_ == NSTK - 1),
                        )

                lag = []
                for st in range(NSTK):
                    sc_ps = [
                        psS.tile([128, 1024], f32, tag="sc", name=f"sc_ps{h2}")
                        for h2 in range(2)
                    ]
                    for h2 in range(2):
                        for tw in range(2):
                            off = h2 * 64
                            tcol = th * 1024 + tw * 512
                            nc.tensor.matmul(
                                sc_ps[h2][:, tw * 512 : (tw + 1) * 512],
                                lhsT=kt_sb[pp][
                                    off : off + 64, st * 128 : (st + 1) * 128
                                ],
                                rhs=qt_sb[pp][off : off + 64, tcol : tcol + 512],
                                start=True,
                                stop=True,
                            )
                    for at_, st_, h2_ in lag:
                        emit_attnv(at_, st_, h2_)
                    lag = []
                    for h2 in range(2):
                        at = at_pool.tile([128, 1024], bf16, tag="at", name="at")
                        nc.scalar.activation(
                            at[:],
                            sc_ps[h2][:],
                            EXP,
                            bias=mb_sb[:, st : st + 1],
                            scale=float(1.0 / np.sqrt(DH)),
                        )
                        if defer_v and st < min(4, NSTK):
                            deferred_at.append((at, st, h2))
                        elif phase_i == len(phases) - 1 and st == NSTK - 1:
                            for a in lag:
                                emit_attnv(*a)
                            lag = []
                            emit_attnv(at, st, h2)
                        else:
                            lag.append((at, st, h2))
                    if defer_av and st == 0:
                        # trailing pp=1 projections: on the PE queue *behind*
                        # this phase's first scores so the EXP chain restarts
                        # ~3.6us sooner after the phase-0 boundary; the av
                        # accumulators allocate after them so the psA ring
                        # stays acyclic (proj slots reuse av_P0's, av_P1
                        # waits only on these chunks' casts).
                        for ch in (
                            (1, 0) + KC[1],
                            (1, 0) + KC[2],
                            (1, 1, 1024, 512),
                            (1, 1, 1536, 512),
                        ):
                            proj_chunk(*ch)
                        av = [
                            [
                                psA.tile(
                                    [128, 512], f32, tag="mm", name=f"av{h2}_{tw}"
                                )
                                for tw in range(2)
                            ]
                            for h2 in range(2)
                        ]
                    if defer_v and st < min(4, NSTK):
                        emit_v_group(st)
                    if defer_v and st == min(4, NSTK) - 1:
                        av = [
                            [
                                psA.tile(
                                    [128, 512], f32, tag="mm", name=f"av{h2}_{tw}"
                                )
                                for tw in range(2)
                            ]
                            for h2 in range(2)
                        ]
                        for at_, st_, h2_ in deferred_at:
                            emit_attnv(at_, st_, h2_)
                        deferred_at = []
                    if st == 1 and pending is not None:
                        emit_normalize(pending)
                    if st == min(5, NSTK - 2) and pending is not None:
                        if pending[1] == 1:
                            emit_wo(pending[0], half=0)
                    if st == min(8, NSTK - 1) and pending is not None:
                        if pending[1] == 1:
                            if min(5, NSTK - 2) < 0:
                                emit_wo(pending[0], half=0)
                            emit_wo(pending[0], half=1)
                        pending = None
                for at_, st_, h2_ in lag:
                    emit_attnv(at_, st_, h2_)
                lag = []
                if phase_i < len(phases) - 1:
                    av_sbs = {}
                    for h2 in range(2):
                        for tw in range(2):
                            av_sb = av_pool.tile(
                                [DH + 1, 512], f32, tag="avsb", name=f"av_sb{h2}_{tw}"
                            )
                            nc.vector.tensor_copy(av_sb[:], av[h2][tw][0 : DH + 1, :])
                            av_sbs[(h2, tw)] = av_sb
                    pending = (th, pp, av_sbs)
                else:
                    pending = (th, pp, av)  # last phase: normalize reads PSUM
                if phase_i == 0:
                    proj_chunk(1, 0, *KC[0])
                    proj_chunk(1, 1, 0, 512)
                    proj_chunk(1, 1, 512, 512)

            # tail: pipeline normalize and Wo by tw halves; fast recip first,
            # all reads straight from the attnV psum (no staging copies).
            # Sums copies split DVE/ACT; y copies split DVE/ACT so the four
            # engines (PE wo, DVE, ACT, GpSimd bcast) drain in parallel.
            th_, pp_, av_ = pending
            sums = sums_pool.tile([97, 512], f32, tag="sums", name="sums")
            nc.gpsimd.memset(sums[:], 1.0)
            for h2 in range(2):
                for tw in range(2):
                    k = 32 * (h2 * 2 + tw)
                    if tw == 0:
                        nc.vector.tensor_copy(
                            sums[k : k + 1, :], av_[h2][tw][DH : DH + 1, :]
                        )
                    else:
                        nc.scalar.copy(
                            sums[k : k + 1, :], av_[h2][tw][DH : DH + 1, :]
                        )
            recips = sums_pool.tile([97, 512], f32, tag="recips", name="recips")
            nc.vector.reciprocal_approx_fast(recips[:], sums[:])
            for tw in range(2):
                for h2 in range(2):
                    k = 32 * (h2 * 2 + tw)
                    tcol = th_ * 1024 + tw * 512
                    r_t = r_pool.tile([1, 512], f32, tag="r", name="r_t")
                    if h2 == 0:
                        nc.vector.tensor_copy(r_t[0:1, :], recips[k : k + 1, :])
                    else:
                        nc.scalar.copy(r_t[0:1, :], recips[k : k + 1, :])
                    rb_t = rb_pool.tile([64, 512], f32, tag="rb", name="rb_t")
                    nc.gpsimd.partition_broadcast(rb_t[:], r_t[0:1, :])
                    nc.vector.tensor_mul(
                        outT_sb[h2 * 64 : (h2 + 1) * 64, pp_, tcol : tcol + 512],
                        av_[h2][tw][0:DH, :],
                        rb_t[:],
                    )
                for i, tt in enumerate(
                    range(th_ * 8 + tw * 4, th_ * 8 + (tw + 1) * 4)
                ):
                    ps = psA.tile([128, 512], f32, tag="mm", name="y_ps")
                    for c in range(HE // 128):
                        nc.tensor.matmul(
                            ps[:],
                            lhsT=outT_sb[:, c, tt * 128 : (tt + 1) * 128],
                            rhs=wo_sb[:, c, :],
                            start=(c == 0),
                            stop=(c == HE // 128 - 1),
                        )
                    y_sb = y_pool.tile([128, 512], f32, tag="y", name="y_sb")
                    if i % 2 == 0:
                        nc.vector.tensor_copy(y_sb[:], ps[:])
                    else:
                        nc.scalar.copy(y_sb[:], ps[:])
                    nc.sync.dma_start(y[tt * 128 : (tt + 1) * 128, :], y_sb[:])

    nc.compile()
    return nc


_NC_CACHE = {}


def _get_nc(SK=1152):
    if SK not in _NC_CACHE:
        _NC_CACHE[SK] = build_nc(SK)
    return _NC_CACHE[SK]


def _pick_sk(mask):
    cnt = int(np.max(mask.sum(axis=1)))
    return max(128, min(S, -(-cnt // 128) * 128))


def make_in_maps(x, mask, Wq, Wk, Wv, Wo, SK):
    bf = ml_dtypes.bfloat16
    xT = np.ascontiguousarray(x.transpose(0, 2, 1)).astype(bf)  # [B, D, S]
    # [H, D, DH] -> [D, H*DH]
    wq_f = np.ascontiguousarray(Wq.transpose(1, 0, 2).reshape(D, H * DH))
    wk_f = np.ascontiguousarray(Wk.transpose(1, 0, 2).reshape(D, H * DH))
    wv_f = np.ascontiguousarray(Wv.transpose(1, 0, 2).reshape(D, H * DH))
    # compacted keys per batch
    xkT = np.zeros((B, D, SK), dtype=bf)
    mbias = np.full((B, SK), -MASK_NUM, dtype=np.float32)
    for b in range(B):
        idx = np.nonzero(mask[b] > 0)[0]
        k = len(idx)
        xkT[b, :, :k] = xT[b][:, idx]
        mbias[b, :k] = 0.0
    in_maps = []
    for c in range(N_CORES):
        b, hg = c // 2, c % 2
        cols = slice(hg * HE, (hg + 1) * HE)
        in_maps.append(
            {
                "xT": xT[b],
                "xkT": xkT[b],
                "wq": np.ascontiguousarray(wq_f[:, cols]).astype(bf),
                "wk": np.ascontiguousarray(wk_f[:, cols]).astype(bf),
                "wv": np.ascontiguousarray(wv_f[:, cols]).astype(bf),
                "wo": np.ascontiguousarray(Wo[cols, :]).astype(bf),
                "mbias": mbias[b],
            }
        )
    return in_maps


def combine_results(results):
    y = np.zeros((B, S, D), np.float32)
    for c in range(N_CORES):
        y[c // 2] += results[c]["y"]
    return y


def kernel(x, mask, Wq, Wk, Wv, Wo):
    mask = np.asarray(mask)
    SK = _pick_sk(mask)
    nc = _get_nc(SK)
    in_maps = make_in_maps(
        np.asarray(x, np.float32),
        mask,
        np.asarray(Wq, np.float32),
        np.asarray(Wk, np.float32),
        np.asarray(Wv, np.float32),
        np.asarray(Wo, np.float32),
        SK,
    )
    res = run_bass_kernel_spmd(nc, in_maps, core_ids=list(range(N_CORES)))
    return combine_results(res.results)


# revision 23
# speedup vs baseline: 1.0050x; 1.0050x over previous
"""Trainium2 Bass kernel for nn_Encoder (B=4, S=2048, D=512, H=8 self-attention).

Sharding over 8 NeuronCores: core c -> (batch b = c//2, head-group hg = c%2).
Each core computes, for its batch and its 4 heads, the full attention block
plus a partial output projection y_part = attn_out @ Wo[group rows]. The host
sums the two partial y tensors per batch (the head-concat + Wo projection is
linear in the head groups).

Key compaction: the key-padding mask zeroes ~half the keys exactly
(exp(-1e9) == 0 in f32), so the host gathers only the unmasked keys per batch
and pads to SK (multiple of 128). Padded keys get kt=0 (score 0) and
mbias=-1e9, so their probs are exactly 0 — identical math, ~44% less
scores/exp/attnV work.

Device-side layout (everything transposed so the contraction dim is always on
SBUF partitions):
  xT [D, S]         : host-pretransposed input, d on partitions (4 chunks)
  xkT [D, SK]       : compacted keys' input columns
  KT per pair       : [128, SK] = [2 heads' e, s], from W.T @ xk matmuls
  QT per pair       : [128, S]
  scoresT [s, t]    : s on partitions -> key-padding mask becomes a
                      per-partition bias AP fused into the ACT Exp instruction
                      (scale=1/sqrt(DH) fused there too)
  V' [s, e + ones]  : appended ones column makes the softmax denominator fall
                      out of the attnV matmul (psum row 64) for free
  outT [he, t]      : exactly the lhsT layout the Wo projection wants
"""

import ml_dtypes
import numpy as np

import concourse.mybir as mybir
import concourse.tile as tile
from concourse import bacc
from concourse.bass_utils import run_bass_kernel_spmd

B, S, D, H = 4, 2048, 512, 8
DH = D // H          # 64
HPC = H // 2         # 4 heads per core
HE = HPC * DH        # 256 output-proj rows per core
T = S                # full query length per core
NDC = D // 128       # 4 contraction chunks for projections
MASK_NUM = 1.0e9
N_CORES = 8

f32 = mybir.dt.float32
bf16 = mybir.dt.bfloat16
EXP = mybir.ActivationFunctionType.Exp


def _chunks(total, width):
    out = []
    o = 0
    while o < total:
        w = min(width, total - o)
        out.append((o, w))
        o += w
    return out


def build_nc(SK):
    NSTK = SK // 128     # key tiles
    nc = bacc.Bacc("TRN2", target_bir_lowering=False, debug=False, num_devices=1)

    xT = nc.dram_tensor("xT", [D, S], bf16, kind="ExternalInput").ap()
    xkT = nc.dram_tensor("xkT", [D, SK], bf16, kind="ExternalInput").ap()
    wq = nc.dram_tensor("wq", [D, HE], bf16, kind="ExternalInput").ap()
    wk = nc.dram_tensor("wk", [D, HE], bf16, kind="ExternalInput").ap()
    wv = nc.dram_tensor("wv", [D, HE], bf16, kind="ExternalInput").ap()
    wo = nc.dram_tensor("wo", [HE, D], bf16, kind="ExternalInput").ap()
    mb = nc.dram_tensor("mbias", [SK], f32, kind="ExternalInput").ap()
    y = nc.dram_tensor("y", [T, D], f32, kind="ExternalOutput").ap()

    with tile.TileContext(nc) as tc:
        with (
            tc.tile_pool(name="const", bufs=1) as const,
            tc.tile_pool(name="psA", bufs=4, space="PSUM") as psA,
            tc.tile_pool(name="psS", bufs=2, space="PSUM") as psS,
            tc.tile_pool(name="attnT", bufs=14) as at_pool,
            tc.tile_pool(name="yout", bufs=3) as y_pool,
            tc.tile_pool(name="recip", bufs=4) as r_pool,
            tc.tile_pool(name="recipb", bufs=4) as rb_pool,
            tc.tile_pool(name="avsb", bufs=8) as av_pool,
            tc.tile_pool(name="sums", bufs=2) as sums_pool,
        ):
            # ---- Stage A: loads -------------------------------------------
            xT_sb = const.tile([128, NDC, S], bf16, tag="xT")
            xk_sb = const.tile([128, NDC, SK], bf16, tag="xk")
            wq_sb = const.tile([128, NDC, HE], bf16, tag="wq")
            wk_sb = const.tile([128, NDC, HE], bf16, tag="wk")
            wv_sb = const.tile([128, NDC, HE], bf16, tag="wv")
            wo_sb = const.tile([128, HE // 128, D], bf16, tag="wo")
            mb_sb = const.tile([128, NSTK], f32, tag="mb")
            xT_r = xT.rearrange("(c p) s -> c p s", p=128)
            xk_r = xkT.rearrange("(c p) s -> c p s", p=128)
            # DMA order: everything the first scores tile + first EXP needs
            # lands first (wk, xk c0, wq, xT c0, mbias), then the rest in
            # roughly the order projection chunks consume it.
            nc.sync.dma_start(wk_sb[:], wk.rearrange("(c p) n -> p c n", p=128))
            for dc in range(NDC):
                nc.sync.dma_start(xk_sb[:, dc, 0:512], xk_r[dc][:, 0:512])
            nc.sync.dma_start(wq_sb[:], wq.rearrange("(c p) n -> p c n", p=128))
            nc.sync.dma_start(mb_sb[:], mb.rearrange("(j p) -> p j", p=128))
            for dc in range(NDC):
                nc.sync.dma_start(xT_sb[:, dc, 0:512], xT_r[dc][:, 0:512])
            nc.sync.dma_start(wv_sb[:], wv.rearrange("(c p) n -> p c n", p=128))
            for off, w in _chunks(SK, 512)[1:]:
                for dc in range(NDC):
                    nc.sync.dma_start(
                        xk_sb[:, dc, off : off + w], xk_r[dc][:, off : off + w]
                    )
            for dc in range(NDC):
                nc.sync.dma_start(xT_sb[:, dc, 512:1024], xT_r[dc][:, 512:1024])
            nc.sync.dma_start(wo_sb[:], wo.rearrange("(c p) n -> p c n", p=128))
            for off in (1024, 1536):
                for dc in range(NDC):
                    nc.sync.dma_start(
                        xT_sb[:, dc, off : off + 512], xT_r[dc][:, off : off + 512]
                    )

            # V' tiles: [s-tile][local head][DH + ones column]
            v_sb = const.tile([128, NSTK, HPC, DH + 1], bf16, tag="v")
            nc.gpsimd.memset(v_sb[:, :, :, DH : DH + 1], 1.0)

            # ---- Stage B: projections -------------------------------------
            # KT per pair pp: [128 (2 heads' e), SK]; QT: [128, S]
            kt_sb = [
                const.tile([128, SK], bf16, tag=f"kt{pp}", name=f"kt{pp}")
                for pp in range(2)
            ]
            qt_sb = [
                const.tile([128, S], bf16, tag=f"qt{pp}", name=f"qt{pp}")
                for pp in range(2)
            ]

            def proj_chunk(pp, which, off, w):
                w_sb, src, dst = (
                    (wk_sb, xk_sb, kt_sb[pp])
                    if which == 0
                    else (wq_sb, xT_sb, qt_sb[pp])
                )
                ps = psA.tile([128, 512], f32, tag="mm", name="proj_ps")
                for dc in range(NDC):
                    nc.tensor.matmul(
                        ps[:, 0:w],
                        lhsT=w_sb[:, dc, pp * 128 : (pp + 1) * 128],
                        rhs=src[:, dc, off : off + w],
                        start=(dc == 0),
                        stop=(dc == NDC - 1),
                    )
                nc.vector.tensor_copy(dst[:, off : off + w], ps[:, 0:w])

            def emit_proj(pp):
                # kt chunk 0 + qt chunks 0,1 unblock the first key tiles;
                # the rest trail and complete while the st loop runs.
                kc = _chunks(SK, 512)
                order = [(0, kc[0]), (1, (0, 512)), (1, (512, 512))]
                order += [(0, c) for c in kc[1:]]
                order += [(1, (1024, 512)), (1, (1536, 512))]
                for which, (off, w) in order:
                    proj_chunk(pp, which, off, w)

            emit_proj(0)

            # V groups: split NSTK tiles into 4 groups emitted at sts 0..3
            base, rem = divmod(NSTK, 4)
            gsz = [base + (1 if g < rem else 0) for g in range(4)]
            goff = [sum(gsz[:g]) for g in range(4)]

            def emit_v_group(g):
                for vst in range(goff[g], goff[g] + gsz[g]):
                    ps = psA.tile([128, HE], f32, tag="mm", name="v_ps")
                    for dc in range(NDC):
                        nc.tensor.matmul(
                            ps[:],
                            lhsT=xk_sb[:, dc, vst * 128 : (vst + 1) * 128],
                            rhs=wv_sb[:, dc, :],
                            start=(dc == 0),
                            stop=(dc == NDC - 1),
                        )
                    nc.vector.tensor_copy(
                        v_sb[:, vst, :, 0:DH],
                        ps[:].rearrange("p (h e) -> p h e", e=DH),
                    )

            # ---- Stage C: attention ---------------------------------------
            # outT [he, t] laid out as [128, 2, T]: chunk pp, rows h2*64.
            # Phase = (th, pp). Normalize + Wo of phase P are deferred into
            # phase P+1 so boundaries never stall PE or ACT. pp=1 projections
            # are emitted at the first phase boundary, off the prologue.
            outT_sb = const.tile([128, HE // 128, T], bf16, tag="outT")

            def emit_normalize(pend):
                th_, pp_, av_sbs_ = pend
                sums = sums_pool.tile([97, 512], f32, tag="sums", name="sums")
                nc.gpsimd.memset(sums[:], 1.0)
                for h2 in range(2):
                    for tw in range(2):
                        k = 32 * (h2 * 2 + tw)
                        nc.vector.tensor_copy(
                            sums[k : k + 1, :],
                            av_sbs_[(h2, tw)][DH : DH + 1, :],
                        )
                recips = sums_pool.tile([97, 512], f32, tag="recips", name="recips")
                nc.vector.reciprocal_approx_fast(recips[:], sums[:])
                for h2 in range(2):
                    for tw in range(2):
                        k = 32 * (h2 * 2 + tw)
                        tcol = th_ * 1024 + tw * 512
                        r_t = r_pool.tile([1, 512], f32, tag="r", name="r_t")
                        nc.vector.tensor_copy(r_t[0:1, :], recips[k : k + 1, :])
                        rb_t = rb_pool.tile([64, 512], f32, tag="rb", name="rb_t")
                        nc.gpsimd.partition_broadcast(rb_t[:], r_t[0:1, :])
                        nc.vector.tensor_mul(
                            outT_sb[h2 * 64 : (h2 + 1) * 64, pp_, tcol : tcol + 512],
                            av_sbs_[(h2, tw)][0:DH, :],
                            rb_t[:],
                        )

            def emit_wo(th_, half=None):
                tts = range(th_ * 8, (th_ + 1) * 8)
                if half is not None:
                    tts = tts[half * 4 : half * 4 + 4]
                for tt in tts:
                    ps = psA.tile([128, 512], f32, tag="mm", name="y_ps")
                    for c in range(HE // 128):
                        nc.tensor.matmul(
                            ps[:],
                            lhsT=outT_sb[:, c, tt * 128 : (tt + 1) * 128],
                            rhs=wo_sb[:, c, :],
                            start=(c == 0),
                            stop=(c == HE // 128 - 1),
                        )
                    y_sb = y_pool.tile([128, 512], f32, tag="y", name="y_sb")
                    nc.vector.tensor_copy(y_sb[:], ps[:])
                    nc.sync.dma_start(y[tt * 128 : (tt + 1) * 128, :], y_sb[:])

            KC = _chunks(SK, 512)
            pending = None
            phases = [(th, pp) for th in range(T // 1024) for pp in range(2)]
            for phase_i, (th, pp) in enumerate(phases):
                defer_v = phase_i == 0
                defer_av = phase_i == 1
                av = None
                if not defer_v and not defer_av:
                    av = [
                        [
                            psA.tile([128, 512], f32, tag="mm", name=f"av{h2}_{tw}")
                            for tw in range(2)
                        ]
                        for h2 in range(2)
                    ]
                deferred_at = []

                def emit_attnv(at_, st_, h2_):
                    h = 2 * pp + h2_
                    for tw in range(2):
                        nc.tensor.matmul(
                            av[h2_][tw][0 : DH + 1, :],
                            lhsT=v_sb[:, st_, h, :],
                            rhs=at_[:, tw * 512 : (tw + 1) * 512],
                            start=(st_ == 0),
                            stop=(st_ == NSTK - 1),
                        )

                lag = []
                for st in range(NSTK):
                    sc_ps = [
                        psS.tile([128, 1024], f32, tag="sc", name=f"sc_ps{h2}")
                        for h2 in range(2)
                    ]
                    for h2 in range(2):
                        for tw in range(2):
                            off = h2 * 64
                            tcol = th * 1024 + tw * 512
                            nc.tensor.matmul(
                                sc_ps[h2][:, tw * 512 : (tw + 1) * 512],
                                lhsT=kt_sb[pp][
                                    off : off + 64, st * 128 : (st + 1) * 128
                                ],
                                rhs=qt_sb[pp][off : off + 64, tcol : tcol + 512],
                                start=True,
                                stop=True,
                            )
                    for at_, st_, h2_ in lag:
                        emit_attnv(at_, st_, h2_)
                    lag = []
                    for h2 in range(2):
                        at = at_pool.tile([128, 1024], bf16, tag="at", name="at")
                        nc.scalar.activation(
                            at[:],
                            sc_ps[h2][:],
                            EXP,
                            bias=mb_sb[:, st : st + 1],
                            scale=float(1.0 / np.sqrt(DH)),
                        )
                        if defer_v and st < min(4, NSTK):
                            deferred_at.append((at, st, h2))
                        else:
                            lag.append((at, st, h2))
                    if defer_av and st == 0:
                        # trailing pp=1 projections: on the PE queue *behind*
                        # this phase's first scores so the EXP chain restarts
                        # ~3.6us sooner after the phase-0 boundary; the av
                        # accumulators allocate after them so the psA ring
                        # stays acyclic (proj slots reuse av_P0's, av_P1
                        # waits only on these chunks' casts).
                        for ch in (
                            (1, 0) + KC[1],
                            (1, 0) + KC[2],
                            (1, 1, 1024, 512),
                            (1, 1, 1536, 512),
                        ):
                            proj_chunk(*ch)
                        av = [
                            [
                                psA.tile(
                                    [128, 512], f32, tag="mm", name=f"av{h2}_{tw}"
                                )
                                for tw in range(2)
                            ]
                            for h2 in range(2)
                        ]
                    if defer_v and st < min(4, NSTK):
                        emit_v_group(st)
                    if defer_v and st == min(4, NSTK) - 1:
                        av = [
                            [
                                psA.tile(
                                    [128, 512], f32, tag="mm", name=f"av{h2}_{tw}"
                                )
                                for tw in range(2)
                            ]
                            for h2 in range(2)
                        ]
                        for at_, st_, h2_ in deferred_at:
                            emit_attnv(at_, st_, h2_)
                        deferred_at = []
                    if st == 1 and pending is not None:
                        emit_normalize(pending)
                    if st == min(5, NSTK - 2) and pending is not None:
                        if pending[1] == 1:
                            emit_wo(pending[0], half=0)
                    if st == min(8, NSTK - 1) and pending is not None:
                        if pending[1] == 1:
                            if min(5, NSTK - 2) < 0:
                                emit_wo(pending[0], half=0)
                            emit_wo(pending[0], half=1)
                        pending = None
                for at_, st_, h2_ in lag:
                    emit_attnv(at_, st_, h2_)
                lag = []
                if phase_i < len(phases) - 1:
                    av_sbs = {}
                    for h2 in range(2):
                        for tw in range(2):
                            av_sb = av_pool.tile(
                                [DH + 1, 512], f32, tag="avsb", name=f"av_sb{h2}_{tw}"
                            )
                            nc.vector.tensor_copy(av_sb[:], av[h2][tw][0 : DH + 1, :])
                            av_sbs[(h2, tw)] = av_sb
                    pending = (th, pp, av_sbs)
                else:
                    pending = (th, pp, av)  # last phase: normalize reads PSUM
                if phase_i == 0:
                    proj_chunk(1, 0, *KC[0])
                    proj_chunk(1, 1, 0, 512)
                    proj_chunk(1, 1, 512, 512)

            # tail: pipeline normalize and Wo by tw halves; fast recip first,
            # all reads straight from the attnV psum (no staging copies).
            # Sums copies split DVE/ACT; y copies split DVE/ACT so the four
            # engines (PE wo, DVE, ACT, GpSimd bcast) drain in parallel.
            th_, pp_, av_ = pending
            sums = sums_pool.tile([97, 512], f32, tag="sums", name="sums")
            nc.gpsimd.memset(sums[:], 1.0)
            for h2 in range(2):
                for tw in range(2):
                    k = 32 * (h2 * 2 + tw)
                    if tw == 0:
                        nc.vector.tensor_copy(
                            sums[k : k + 1, :], av_[h2][tw][DH : DH + 1, :]
                        )
                    else:
                        nc.scalar.copy(
                            sums[k : k + 1, :], av_[h2][tw][DH : DH + 1, :]
                        )
            recips = sums_pool.tile([97, 512], f32, tag="recips", name="recips")
            nc.vector.reciprocal_approx_fast(recips[:], sums[:])
            for tw in range(2):
                for h2 in range(2):
                    k = 32 * (h2 * 2 + tw)
                    tcol = th_ * 1024 + tw * 512
                    r_t = r_pool.tile([1, 512], f32, tag="r", name="r_t")
                    if h2 == 0:
                        nc.vector.tensor_copy(r_t[0:1, :], recips[k : k + 1, :])
                    else:
                        nc.scalar.copy(r_t[0:1, :], recips[k : k + 1, :])
                    rb_t = rb_pool.tile([64, 512], f32, tag="rb", name="rb_t")
                    nc.gpsimd.partition_broadcast(rb_t[:], r_t[0:1, :])
                    nc.vector.tensor_mul(
                        outT_sb[h2 * 64 : (h2 + 1) * 64, pp_, tcol : tcol + 512],
                        av_[h2][tw][0:DH, :],
                        rb_t[:],
                    )
                for i, tt in enumerate(
                    range(th_ * 8 + tw * 4, th_ * 8 + (tw + 1) * 4)
                ):
                    ps = psA.tile([128, 512], f32, tag="mm", name="y_ps")
                    for c in range(HE // 128):
                        nc.tensor.matmul(
                            ps[:],
                            lhsT=outT_sb[:, c, tt * 128 : (tt + 1) * 128],
                            rhs=wo_sb[:, c, :],
                            start=(c == 0),
                            stop=(c == HE // 128 - 1),
                        )
                    y_sb = y_pool.tile([128, 512], f32, tag="y", name="y_sb")
                    if i % 2 == 0:
                        nc.vector.tensor_copy(y_sb[:], ps[:])
                    else:
                        nc.scalar.copy(y_sb[:], ps[:])
                    nc.sync.dma_start(y[tt * 128 : (tt + 1) * 128, :], y_sb[:])

    nc.compile()
    return nc


_NC_CACHE = {}


def _get_nc(SK=1152):
    if SK not in _NC_CACHE:
        _NC_CACHE[SK] = build_nc(SK)
    return _NC_CACHE[SK]


def _pick_sk(mask):
    cnt = int(np.max(mask.sum(axis=1)))
    return max(128, min(S, -(-cnt // 128) * 128))


def make_in_maps(x, mask, Wq, Wk, Wv, Wo, SK):
    bf = ml_dtypes.bfloat16
    xT = np.ascontiguousarray(x.transpose(0, 2, 1)).astype(bf)  # [B, D, S]
    # [H, D, DH] -> [D, H*DH]
    wq_f = np.ascontiguousarray(Wq.transpose(1, 0, 2).reshape(D, H * DH))
    wk_f = np.ascontiguousarray(Wk.transpose(1, 0, 2).reshape(D, H * DH))
    wv_f = np.ascontiguousarray(Wv.transpose(1, 0, 2).reshape(D, H * DH))
    # compacted keys per batch
    xkT = np.zeros((B, D, SK), dtype=bf)
    mbias = np.full((B, SK), -MASK_NUM, dtype=np.float32)
    for b in range(B):
        idx = np.nonzero(mask[b] > 0)[0]
        k = len(idx)
        xkT[b, :, :k] = xT[b][:, idx]
        mbias[b, :k] = 0.0
    in_maps = []
    for c in range(N_CORES):
        b, hg = c // 2, c % 2
        cols = slice(hg * HE, (hg + 1) * HE)
        in_maps.append(
            {
                "xT": xT[b],
                "xkT": xkT[b],
                "wq": np.ascontiguousarray(wq_f[:, cols]).astype(bf),
                "wk": np.ascontiguousarray(wk_f[:, cols]).astype(bf),
                "wv": np.ascontiguousarray(wv_f[:, cols]).astype(bf),
                "wo": np.ascontiguousarray(Wo[cols, :]).astype(bf),
                "mbias": mbias[b],
            }
        )
    return in_maps


def combine_results(results):
    y = np.zeros((B, S, D), np.float32)
    for c in range(N_CORES):
        y[c // 2] += results[c]["y"]
    return y


def kernel(x, mask, Wq, Wk, Wv, Wo):
    mask = np.asarray(mask)
    SK = _pick_sk(mask)
    nc = _get_nc(SK)
    in_maps = make_in_maps(
        np.asarray(x, np.float32),
        mask,
        np.asarray(Wq, np.float32),
        np.asarray(Wk, np.float32),
        np.asarray(Wv, np.float32),
        np.asarray(Wo, np.float32),
        SK,
    )
    res = run_bass_kernel_spmd(nc, in_maps, core_ids=list(range(N_CORES)))
    return combine_results(res.results)


# revision 24
# speedup vs baseline: 1.0294x; 1.0243x over previous
"""Trainium2 Bass kernel for nn_Encoder (B=4, S=2048, D=512, H=8 self-attention).

Sharding over 8 NeuronCores: core c -> (batch b = c//2, head-group hg = c%2).
Each core computes, for its batch and its 4 heads, the full attention block
plus a partial output projection y_part = attn_out @ Wo[group rows]. The host
sums the two partial y tensors per batch (the head-concat + Wo projection is
linear in the head groups).

Key compaction: the key-padding mask zeroes ~half the keys exactly
(exp(-1e9) == 0 in f32), so the host gathers only the unmasked keys per batch
and pads to SK (multiple of 128). Padded keys get kt=0 (score 0) and
mbias=-1e9, so their probs are exactly 0 — identical math, ~44% less
scores/exp/attnV work.

Device-side layout (everything transposed so the contraction dim is always on
SBUF partitions):
  xT [D, S]         : host-pretransposed input, d on partitions (4 chunks)
  xkT [D, SK]       : compacted keys' input columns
  KT per pair       : [128, SK] = [2 heads' e, s], from W.T @ xk matmuls
  QT per pair       : [128, S]
  scoresT [s, t]    : s on partitions -> key-padding mask becomes a
                      per-partition bias AP fused into the ACT Exp instruction
                      (scale=1/sqrt(DH) fused there too)
  V' [s, e + ones]  : appended ones column makes the softmax denominator fall
                      out of the attnV matmul (psum row 64) for free
  outT [he, t]      : exactly the lhsT layout the Wo projection wants
"""

import ml_dtypes
import numpy as np

import concourse.mybir as mybir
import concourse.tile as tile
from concourse import bacc
from concourse.bass_utils import run_bass_kernel_spmd

B, S, D, H = 4, 2048, 512, 8
DH = D // H          # 64
HPC = H // 2         # 4 heads per core
HE = HPC * DH        # 256 output-proj rows per core
T = S                # full query length per core
NDC = D // 128       # 4 contraction chunks for projections
MASK_NUM = 1.0e9
N_CORES = 8

f32 = mybir.dt.float32
bf16 = mybir.dt.bfloat16
EXP = mybir.ActivationFunctionType.Exp


def _chunks(total, width):
    out = []
    o = 0
    while o < total:
        w = min(width, total - o)
        out.append((o, w))
        o += w
    return out


def build_nc(SK):
    NSTK = SK // 128     # key tiles
    nc = bacc.Bacc("TRN2", target_bir_lowering=False, debug=False, num_devices=1)

    xT = nc.dram_tensor("xT", [D, S], bf16, kind="ExternalInput").ap()
    xkT = nc.dram_tensor("xkT", [D, SK], bf16, kind="ExternalInput").ap()
    wq = nc.dram_tensor("wq", [D, HE], bf16, kind="ExternalInput").ap()
    wk = nc.dram_tensor("wk", [D, HE], bf16, kind="ExternalInput").ap()
    wv = nc.dram_tensor("wv", [D, HE], bf16, kind="ExternalInput").ap()
    wo = nc.dram_tensor("wo", [HE, D], bf16, kind="ExternalInput").ap()
    mb = nc.dram_tensor("mbias", [SK], f32, kind="ExternalInput").ap()
    y = nc.dram_tensor("y", [T, D], f32, kind="ExternalOutput").ap()

    with tile.TileContext(nc) as tc:
        with (
            tc.tile_pool(name="const", bufs=1) as const,
            tc.tile_pool(name="psA", bufs=4, space="PSUM") as psA,
            tc.tile_pool(name="psS", bufs=2, space="PSUM") as psS,
            tc.tile_pool(name="attnT", bufs=16) as at_pool,
            tc.tile_pool(name="yout", bufs=6) as y_pool,
            tc.tile_pool(name="recip", bufs=8) as r_pool,
            tc.tile_pool(name="recipb", bufs=8) as rb_pool,
            tc.tile_pool(name="avsb", bufs=8) as av_pool,
            tc.tile_pool(name="sums", bufs=2) as sums_pool,
        ):
            # ---- Stage A: loads -------------------------------------------
            xT_sb = const.tile([128, NDC, S], bf16, tag="xT")
            xk_sb = const.tile([128, NDC, SK], bf16, tag="xk")
            wq_sb = const.tile([128, NDC, HE], bf16, tag="wq")
            wk_sb = const.tile([128, NDC, HE], bf16, tag="wk")
            wv_sb = const.tile([128, NDC, HE], bf16, tag="wv")
            wo_sb = const.tile([128, HE // 128, D], bf16, tag="wo")
            mb_sb = const.tile([128, NSTK], f32, tag="mb")
            xT_r = xT.rearrange("(c p) s -> c p s", p=128)
            xk_r = xkT.rearrange("(c p) s -> c p s", p=128)
            # DMA order: everything the first scores tile + first EXP needs
            # lands first (wk, xk c0, wq, xT c0, mbias), then the rest in
            # roughly the order projection chunks consume it.
            nc.sync.dma_start(wk_sb[:], wk.rearrange("(c p) n -> p c n", p=128))
            for dc in range(NDC):
                nc.sync.dma_start(xk_sb[:, dc, 0:512], xk_r[dc][:, 0:512])
            nc.sync.dma_start(wq_sb[:], wq.rearrange("(c p) n -> p c n", p=128))
            nc.sync.dma_start(mb_sb[:], mb.rearrange("(j p) -> p j", p=128))
            for dc in range(NDC):
                nc.sync.dma_start(xT_sb[:, dc, 0:512], xT_r[dc][:, 0:512])
            nc.sync.dma_start(wv_sb[:], wv.rearrange("(c p) n -> p c n", p=128))
            for off, w in _chunks(SK, 512)[1:]:
                for dc in range(NDC):
                    nc.sync.dma_start(
                        xk_sb[:, dc, off : off + w], xk_r[dc][:, off : off + w]
                    )
            for dc in range(NDC):
                nc.sync.dma_start(xT_sb[:, dc, 512:1024], xT_r[dc][:, 512:1024])
            nc.sync.dma_start(wo_sb[:], wo.rearrange("(c p) n -> p c n", p=128))
            for off in (1024, 1536):
                for dc in range(NDC):
                    nc.sync.dma_start(
                        xT_sb[:, dc, off : off + 512], xT_r[dc][:, off : off + 512]
                    )

            # V' tiles: [s-tile][local head][DH + ones column]
            v_sb = const.tile([128, NSTK, HPC, DH + 1], bf16, tag="v")
            nc.gpsimd.memset(v_sb[:, :, :, DH : DH + 1], 1.0)

            # ---- Stage B: projections -------------------------------------
            # KT per pair pp: [128 (2 heads' e), SK]; QT: [128, S]
            kt_sb = [
                const.tile([128, SK], bf16, tag=f"kt{pp}", name=f"kt{pp}")
                for pp in range(2)
            ]
            qt_sb = [
                const.tile([128, S], bf16, tag=f"qt{pp}", name=f"qt{pp}")
                for pp in range(2)
            ]

            def proj_chunk(pp, which, off, w):
                w_sb, src, dst = (
                    (wk_sb, xk_sb, kt_sb[pp])
                    if which == 0
                    else (wq_sb, xT_sb, qt_sb[pp])
                )
                ps = psA.tile([128, 512], f32, tag="mm", name="proj_ps")
                for dc in range(NDC):
                    nc.tensor.matmul(
                        ps[:, 0:w],
                        lhsT=w_sb[:, dc, pp * 128 : (pp + 1) * 128],
                        rhs=src[:, dc, off : off + w],
                        start=(dc == 0),
                        stop=(dc == NDC - 1),
                    )
                nc.vector.tensor_copy(dst[:, off : off + w], ps[:, 0:w])

            def emit_proj(pp):
                # kt chunk 0 + qt chunks 0,1 unblock the first key tiles;
                # the rest trail and complete while the st loop runs.
                kc = _chunks(SK, 512)
                order = [(0, kc[0]), (1, (0, 512)), (1, (512, 512))]
                order += [(0, c) for c in kc[1:]]
                order += [(1, (1024, 512)), (1, (1536, 512))]
                for which, (off, w) in order:
                    proj_chunk(pp, which, off, w)

            emit_proj(0)

            # V groups: split NSTK tiles into 4 groups emitted at sts 0..3
            base, rem = divmod(NSTK, 4)
            gsz = [base + (1 if g < rem else 0) for g in range(4)]
            goff = [sum(gsz[:g]) for g in range(4)]

            def emit_v_group(g):
                for vst in range(goff[g], goff[g] + gsz[g]):
                    ps = psA.tile([128, HE], f32, tag="mm", name="v_ps")
                    for dc in range(NDC):
                        nc.tensor.matmul(
                            ps[:],
                            lhsT=xk_sb[:, dc, vst * 128 : (vst + 1) * 128],
                            rhs=wv_sb[:, dc, :],
                            start=(dc == 0),
                            stop=(dc == NDC - 1),
                        )
                    nc.vector.tensor_copy(
                        v_sb[:, vst, :, 0:DH],
                        ps[:].rearrange("p (h e) -> p h e", e=DH),
                    )

            # ---- Stage C: attention ---------------------------------------
            # outT [he, t] laid out as [128, 2, T]: chunk pp, rows h2*64.
            # Phase = (th, pp). Normalize + Wo of phase P are deferred into
            # phase P+1 so boundaries never stall PE or ACT. pp=1 projections
            # are emitted at the first phase boundary, off the prologue.
            outT_sb = const.tile([128, HE // 128, T], bf16, tag="outT")

            def emit_normalize(pend):
                th_, pp_, av_sbs_ = pend
                sums = sums_pool.tile([97, 512], f32, tag="sums", name="sums")
                nc.gpsimd.memset(sums[:], 1.0)
                for h2 in range(2):
                    for tw in range(2):
                        k = 32 * (h2 * 2 + tw)
                        nc.vector.tensor_copy(
                            sums[k : k + 1, :],
                            av_sbs_[(h2, tw)][DH : DH + 1, :],
                        )
                recips = sums_pool.tile([97, 512], f32, tag="recips", name="recips")
                nc.vector.reciprocal_approx_fast(recips[:], sums[:])
                for h2 in range(2):
                    for tw in range(2):
                        k = 32 * (h2 * 2 + tw)
                        tcol = th_ * 1024 + tw * 512
                        r_t = r_pool.tile([1, 512], f32, tag="r", name="r_t")
                        nc.vector.tensor_copy(r_t[0:1, :], recips[k : k + 1, :])
                        rb_t = rb_pool.tile([64, 512], f32, tag="rb", name="rb_t")
                        nc.gpsimd.partition_broadcast(rb_t[:], r_t[0:1, :])
                        nc.vector.tensor_mul(
                            outT_sb[h2 * 64 : (h2 + 1) * 64, pp_, tcol : tcol + 512],
                            av_sbs_[(h2, tw)][0:DH, :],
                            rb_t[:],
                        )

            def emit_wo(th_, half=None):
                tts = range(th_ * 8, (th_ + 1) * 8)
                if half is not None:
                    tts = tts[half * 4 : half * 4 + 4]
                for tt in tts:
                    ps = psA.tile([128, 512], f32, tag="mm", name="y_ps")
                    for c in range(HE // 128):
                        nc.tensor.matmul(
                            ps[:],
                            lhsT=outT_sb[:, c, tt * 128 : (tt + 1) * 128],
                            rhs=wo_sb[:, c, :],
                            start=(c == 0),
                            stop=(c == HE // 128 - 1),
                        )
                    y_sb = y_pool.tile([128, 512], f32, tag="y", name="y_sb")
                    nc.vector.tensor_copy(y_sb[:], ps[:])
                    nc.sync.dma_start(y[tt * 128 : (tt + 1) * 128, :], y_sb[:])

            KC = _chunks(SK, 512)
            pending = None
            phases = [(th, pp) for th in range(T // 1024) for pp in range(2)]
            for phase_i, (th, pp) in enumerate(phases):
                defer_v = phase_i == 0
                defer_av = phase_i == 1
                av = None
                if not defer_v and not defer_av:
                    av = [
                        [
                            psA.tile([128, 512], f32, tag="mm", name=f"av{h2}_{tw}")
                            for tw in range(2)
                        ]
                        for h2 in range(2)
                    ]
                deferred_at = []

                def emit_attnv(at_, st_, h2_):
                    h = 2 * pp + h2_
                    for tw in range(2):
                        nc.tensor.matmul(
                            av[h2_][tw][0 : DH + 1, :],
                            lhsT=v_sb[:, st_, h, :],
                            rhs=at_[:, tw * 512 : (tw + 1) * 512],
                            start=(st_ == 0),
                            stop=(st_ == NSTK - 1),
                        )

                lag = []
                for st in range(NSTK):
                    sc_ps = [
                        psS.tile([128, 1024], f32, tag="sc", name=f"sc_ps{h2}")
                        for h2 in range(2)
                    ]
                    for h2 in range(2):
                        for tw in range(2):
                            off = h2 * 64
                            tcol = th * 1024 + tw * 512
                            nc.tensor.matmul(
                                sc_ps[h2][:, tw * 512 : (tw + 1) * 512],
                                lhsT=kt_sb[pp][
                                    off : off + 64, st * 128 : (st + 1) * 128
                                ],
                                rhs=qt_sb[pp][off : off + 64, tcol : tcol + 512],
                                start=True,
                                stop=True,
                            )
                    for at_, st_, h2_ in lag:
                        emit_attnv(at_, st_, h2_)
                    lag = []
                    for h2 in range(2):
                        at = at_pool.tile([128, 1024], bf16, tag="at", name="at")
                        nc.scalar.activation(
                            at[:],
                            sc_ps[h2][:],
                            EXP,
                            bias=mb_sb[:, st : st + 1],
                            scale=float(1.0 / np.sqrt(DH)),
                        )
                        if defer_v and st < min(4, NSTK):
                            deferred_at.append((at, st, h2))
                        else:
                            lag.append((at, st, h2))
                    if defer_av and st == 0:
                        # trailing pp=1 projections: on the PE queue *behind*
                        # this phase's first scores so the EXP chain restarts
                        # ~3.6us sooner after the phase-0 boundary; the av
                        # accumulators allocate after them so the psA ring
                        # stays acyclic (proj slots reuse av_P0's, av_P1
                        # waits only on these chunks' casts).
                        for ch in (
                            (1, 0) + KC[1],
                            (1, 0) + KC[2],
                            (1, 1, 1024, 512),
                            (1, 1, 1536, 512),
                        ):
                            proj_chunk(*ch)
                        av = [
                            [
                                psA.tile(
                                    [128, 512], f32, tag="mm", name=f"av{h2}_{tw}"
                                )
                                for tw in range(2)
                            ]
                            for h2 in range(2)
                        ]
                    if defer_v and st < min(4, NSTK):
                        emit_v_group(st)
                    if defer_v and st == min(4, NSTK) - 1:
                        av = [
                            [
                                psA.tile(
                                    [128, 512], f32, tag="mm", name=f"av{h2}_{tw}"
                                )
                                for tw in range(2)
                            ]
                            for h2 in range(2)
                        ]
                        for at_, st_, h2_ in deferred_at:
                            emit_attnv(at_, st_, h2_)
                        deferred_at = []
                    if st == 1 and pending is not None:
                        emit_normalize(pending)
                    if st == min(5, NSTK - 2) and pending is not None:
                        if pending[1] == 1:
                            emit_wo(pending[0], half=0)
                    if st == min(8, NSTK - 1) and pending is not None:
                        if pending[1] == 1:
                            if min(5, NSTK - 2) < 0:
                                emit_wo(pending[0], half=0)
                            emit_wo(pending[0], half=1)
                        pending = None
                for at_, st_, h2_ in lag:
                    emit_attnv(at_, st_, h2_)
                lag = []
                if phase_i < len(phases) - 1:
                    av_sbs = {}
                    for h2 in range(2):
                        for tw in range(2):
                            av_sb = av_pool.tile(
                                [DH + 1, 512], f32, tag="avsb", name=f"av_sb{h2}_{tw}"
                            )
                            nc.vector.tensor_copy(av_sb[:], av[h2][tw][0 : DH + 1, :])
                            av_sbs[(h2, tw)] = av_sb
                    pending = (th, pp, av_sbs)
                else:
                    pending = (th, pp, av)  # last phase: normalize reads PSUM
                if phase_i == 0:
                    proj_chunk(1, 0, *KC[0])
                    proj_chunk(1, 1, 0, 512)
                    proj_chunk(1, 1, 512, 512)

            # tail: pipeline normalize and Wo by tw halves; fast recip first,
            # all reads straight from the attnV psum (no staging copies).
            # Sums copies split DVE/ACT; y copies split DVE/ACT so the four
            # engines (PE wo, DVE, ACT, GpSimd bcast) drain in parallel.
            th_, pp_, av_ = pending
            sums = sums_pool.tile([97, 512], f32, tag="sums", name="sums")
            nc.gpsimd.memset(sums[:], 1.0)
            for h2 in range(2):
                for tw in range(2):
                    k = 32 * (h2 * 2 + tw)
                    if tw == 0:
                        nc.vector.tensor_copy(
                            sums[k : k + 1, :], av_[h2][tw][DH : DH + 1, :]
                        )
                    else:
                        nc.scalar.copy(
                            sums[k : k + 1, :], av_[h2][tw][DH : DH + 1, :]
                        )
            recips = sums_pool.tile([97, 512], f32, tag="recips", name="recips")
            nc.vector.reciprocal_approx_fast(recips[:], sums[:])
            for tw in range(2):
                for h2 in range(2):
                    k = 32 * (h2 * 2 + tw)
                    tcol = th_ * 1024 + tw * 512
                    r_t = r_pool.tile([1, 512], f32, tag="r", name="r_t")
                    if h2 == 0:
                        nc.vector.tensor_copy(r_t[0:1, :], recips[k : k + 1, :])
                    else:
                        nc.scalar.copy(r_t[0:1, :], recips[k : k + 1, :])
                    rb_t = rb_pool.tile([64, 512], f32, tag="rb", name="rb_t")
                    nc.gpsimd.partition_broadcast(rb_t[:], r_t[0:1, :])
                    nc.vector.tensor_mul(
                        outT_sb[h2 * 64 : (h2 + 1) * 64, pp_, tcol : tcol + 512],
                        av_[h2][tw][0:DH, :],
                        rb_t[:],
                    )
                for i, tt in enumerate(
                    range(th_ * 8 + tw * 4, th_ * 8 + (tw + 1) * 4)
                ):
                    ps = psA.tile([128, 512], f32, tag="mm", name="y_ps")
                    for c in range(HE // 128):
                        nc.tensor.matmul(
                            ps[:],
                            lhsT=outT_sb[:, c, tt * 128 : (tt + 1) * 128],
                            rhs=wo_sb[:, c, :],
                            start=(c == 0),
                            stop=(c == HE // 128 - 1),
                        )
                    y_sb = y_pool.tile([128, 512], f32, tag="y", name="y_sb")
                    if i % 2 == 0:
                        nc.vector.tensor_copy(y_sb[:], ps[:])
                    else:
                        nc.scalar.copy(y_sb[:], ps[:])
                    nc.sync.dma_start(y[tt * 128 : (tt + 1) * 128, :], y_sb[:])

    nc.compile()
    return nc


_NC_CACHE = {}


def _get_nc(SK=1152):
    if SK not in _NC_CACHE:
        _NC_CACHE[SK] = build_nc(SK)
    return _NC_CACHE[SK]


def _pick_sk(mask):
    cnt = int(np.max(mask.sum(axis=1)))
    return max(128, min(S, -(-cnt // 128) * 128))


def make_in_maps(x, mask, Wq, Wk, Wv, Wo, SK):
    bf = ml_dtypes.bfloat16
    xT = np.ascontiguousarray(x.transpose(0, 2, 1)).astype(bf)  # [B, D, S]
    # [H, D, DH] -> [D, H*DH]
    wq_f = np.ascontiguousarray(Wq.transpose(1, 0, 2).reshape(D, H * DH))
    wk_f = np.ascontiguousarray(Wk.transpose(1, 0, 2).reshape(D, H * DH))
    wv_f = np.ascontiguousarray(Wv.transpose(1, 0, 2).reshape(D, H * DH))
    # compacted keys per batch
    xkT = np.zeros((B, D, SK), dtype=bf)
    mbias = np.full((B, SK), -MASK_NUM, dtype=np.float32)
    for b in range(B):
        idx = np.nonzero(mask[b] > 0)[0]
        k = len(idx)
        xkT[b, :, :k] = xT[b][:, idx]
        mbias[b, :k] = 0.0
    in_maps = []
    for c in range(N_CORES):
        b, hg = c // 2, c % 2
        cols = slice(hg * HE, (hg + 1) * HE)
        in_maps.append(
            {
                "xT": xT[b],
                "xkT": xkT[b],
                "wq": np.ascontiguousarray(wq_f[:, cols]).astype(bf),
                "wk": np.ascontiguousarray(wk_f[:, cols]).astype(bf),
                "wv": np.ascontiguousarray(wv_f[:, cols]).astype(bf),
                "wo": np.ascontiguousarray(Wo[cols, :]).astype(bf),
                "mbias": mbias[b],
            }
        )
    return in_maps


def combine_results(results):
    y = np.zeros((B, S, D), np.float32)
    for c in range(N_CORES):
        y[c // 2] += results[c]["y"]
    return y


def kernel(x, mask, Wq, Wk, Wv, Wo):
    mask = np.asarray(mask)
    SK = _pick_sk(mask)
    nc = _get_nc(SK)
    in_maps = make_in_maps(
        np.asarray(x, np.float32),
        mask,
        np.asarray(Wq, np.float32),
        np.asarray(Wk, np.float32),
        np.asarray(Wv, np.float32),
        np.asarray(Wo, np.float32),
        SK,
    )
    res = run_bass_kernel_spmd(nc, in_maps, core_ids=list(range(N_CORES)))
    return combine_results(res.results)
